# revision 20
# baseline (speedup 1.0000x reference)
"""Trainium2 Bass kernel for nn_MultiModalFusion (moe_routing).

Strategy:
- Pure data-parallel over 8 cores; host sorts samples by expert label so each
  core sees 4 contiguous expert groups of fixed capacity (static shapes, only
  1 of 4 expert matmuls runs per sample).
- Feature-partitioned ("transposed") layout on device: activations are
  [feature, sample]; all dense math is weight-stationary fp32r matmuls
  (1 cyc/row for N>=256, ~1.5e-4 rel err).
- out_proj is folded into fus_w1 on the host (saves a full [B*3,512]x[512,512]
  matmul); the 1/sqrt(hd) score scale is folded into W_q.
- Tiny-seq (3 tokens) attention: q*k products on DVE (bf16, 2x mode);
  per-head d-reduction and softmax probability broadcast-over-d are done with
  small constant selection-matrix matmuls on the tensor engine.
"""

import numpy as np

import concourse.bass as bass
import concourse.mybir as mybir
import concourse.tile as tile
from concourse import bacc
from concourse.bass_utils import run_bass_kernel_spmd

E = 512
H = 256
NH = 8
HD = 64
NE = 4
B = 16384
NCORES = 8
CAP = 544            # per-core per-expert capacity (ceil(B/4/8) + slack)
R = NE * CAP         # 2176 columns per core
C = 272              # chunk = half an expert group; 8 chunks, all N>=256
NCH = R // C

LAST_RESULTS = None  # BassKernelResults of the most recent kernel() call
LAST_NC = None       # finalized Bass program of the most recent kernel() call

F32 = mybir.dt.float32
F32R = mybir.dt.float32r
BF16 = mybir.dt.bfloat16
AF = mybir.ActivationFunctionType
ALU = mybir.AluOpType


def _build_program():
    nc = bacc.Bacc("TRN2")

    # ---------------- DRAM I/O ----------------
    xt = nc.dram_tensor("xt", [3, 4, 128, R], F32R, kind="ExternalInput")
    wqkv = nc.dram_tensor("wqkv", [128, 4, 1536], F32R, kind="ExternalInput")
    bqkv = nc.dram_tensor("bqkv", [128, 12], F32, kind="ExternalInput")
    w1o = nc.dram_tensor("w1o", [128, 12, 256], F32R, kind="ExternalInput")
    beff = nc.dram_tensor("beff", [128, 2], F32, kind="ExternalInput")
    w2 = nc.dram_tensor("w2", [128, 2, 512], F32R, kind="ExternalInput")
    b2 = nc.dram_tensor("b2", [128, 4], F32, kind="ExternalInput")
    lng = nc.dram_tensor("lng", [128, 4], F32, kind="ExternalInput")
    lnb = nc.dram_tensor("lnb", [128, 4], F32, kind="ExternalInput")
    waff = nc.dram_tensor("waff", [128, 4, 2048], F32R, kind="ExternalInput")
    baff = nc.dram_tensor("baff", [128, 16], F32, kind="ExternalInput")
    sel = nc.dram_tensor("sel", [128, 4, 8], BF16, kind="ExternalInput")
    exps = nc.dram_tensor("exps", [8, 4, 128], F32R, kind="ExternalInput")
    ones512 = nc.dram_tensor("ones512", [128, 1], F32R, kind="ExternalInput")
    onesk1 = nc.dram_tensor("onesk1", [1, 128], F32R, kind="ExternalInput")
    outT = nc.dram_tensor("outT", [4, 128, R], F32, kind="ExternalOutput")

    with tile.TileContext(nc) as tc:
        with tc.tile_pool(name="wp", bufs=1) as wp, \
             tc.tile_pool(name="xp", bufs=2) as xp, \
             tc.tile_pool(name="ap", bufs=1) as ap, \
             tc.tile_pool(name="sp", bufs=1) as sp, \
             tc.tile_pool(name="sp2", bufs=2) as sp2, \
             tc.tile_pool(name="psA", bufs=4, space="PSUM") as psA, \
             tc.tile_pool(name="psS", bufs=2, space="PSUM") as psS, \
             tc.tile_pool(name="psT", bufs=2, space="PSUM") as psT:

            # ---------------- load weights/constants once ----------------
            wqkv_sb = wp.tile([128, 4, 1536], F32R)
            nc.sync.dma_start(wqkv_sb[:], wqkv[:])
            bqkv_sb = wp.tile([128, 12], F32)
            nc.sync.dma_start(bqkv_sb[:], bqkv[:])
            w1o_sb = wp.tile([128, 12, 256], F32R)
            nc.sync.dma_start(w1o_sb[:], w1o[:])
            beff_sb = wp.tile([128, 2], F32)
            nc.sync.dma_start(beff_sb[:], beff[:])
            w2_sb = wp.tile([128, 2, 512], F32R)
            nc.sync.dma_start(w2_sb[:], w2[:])
            b2_sb = wp.tile([128, 4], F32)
            nc.sync.dma_start(b2_sb[:], b2[:])
            lng_sb = wp.tile([128, 4], F32)
            nc.sync.dma_start(lng_sb[:], lng[:])
            lnb_sb = wp.tile([128, 4], F32)
            nc.sync.dma_start(lnb_sb[:], lnb[:])
            waff_sb = wp.tile([128, 4, 2048], F32R)
            nc.sync.dma_start(waff_sb[:], waff[:])
            baff_sb = wp.tile([128, 16], F32)
            nc.sync.dma_start(baff_sb[:], baff[:])
            sel_sb = wp.tile([128, 4, 8], BF16)
            nc.sync.dma_start(sel_sb[:], sel[:])
            exps_sb = wp.tile([8, 4, 128], F32R)
            nc.sync.dma_start(exps_sb[:], exps[:])
            o512_sb = wp.tile([128, 1], F32R)
            nc.sync.dma_start(o512_sb[:], ones512[:])
            ok1_sb = wp.tile([1, 128], F32R)
            nc.sync.dma_start(ok1_sb[:], onesk1[:])
            eps_sb = wp.tile([1, 1], F32)
            nc.vector.memset(eps_sb[:], 1e-5)

            for ch in range(NCH):
                col = ch * C
                exp_idx = ch // 2  # expert for this chunk

                # ---------------- load x chunk ----------------
                x_sb = xp.tile([128, 3, 4, C], F32R, tag="x")
                for t in range(3):
                    for p in range(4):
                        nc.sync.dma_start(x_sb[:, t, p, :],
                                          xt[t, p, :, col:col + C])

                # ---------------- QKV projection ----------------
                q_sb = ap.tile([128, 3, 4, C], BF16, tag="q")
                k_sb = ap.tile([128, 3, 4, C], BF16, tag="k")
                v_sb = ap.tile([128, 3, 4, C], F32R, tag="v")
                for t in range(3):
                    for mi in range(12):
                        qp = psA.tile([128, C], F32, tag="mm", name=f"qkv{ch}_{t}_{mi}")
                        for ks in range(4):
                            nc.tensor.matmul(
                                qp[:],
                                wqkv_sb[:, ks, mi * 128:(mi + 1) * 128],
                                x_sb[:, t, ks, :],
                                start=(ks == 0), stop=(ks == 3))
                        dst = (q_sb, k_sb, v_sb)[mi // 4]
                        nc.scalar.activation(
                            dst[:, t, mi % 4, :], qp[:], AF.Identity,
                            bias=bqkv_sb[:, mi:mi + 1], scale=1.0)

                # ---------------- attention scores ----------------
                e_sb = sp.tile([8, 3, 3, C], F32, tag="esb")
                for i in range(3):
                    for j in range(3):
                        prod = sp2.tile([128, 4, C], BF16, tag="prod",
                                        name=f"prod{ch}_{i}_{j}")
                        nc.vector.tensor_tensor(
                            prod[:], q_sb[:, i, :, :], k_sb[:, j, :, :],
                            ALU.mult)
                        s_ps = psS.tile([8, C], F32, tag="s",
                                        name=f"s{ch}_{i}_{j}")
                        for p in range(4):
                            nc.tensor.matmul(
                                s_ps[:], sel_sb[:, p, :], prod[:, p, :],
                                start=(p == 0), stop=(p == 3))
                        nc.scalar.activation(
                            e_sb[:, i, j, :], s_ps[:], AF.Exp)

                # ---------------- softmax over j ----------------
                z_sb = sp.tile([8, 3, C], F32, tag="z")
                nc.vector.tensor_tensor(z_sb[:], e_sb[:, :, 0, :],
                                        e_sb[:, :, 1, :], ALU.add)
                nc.vector.tensor_tensor(z_sb[:], z_sb[:], e_sb[:, :, 2, :],
                                        ALU.add)
                r_sb = sp.tile([8, 3, C], F32, tag="r")
                nc.vector.reciprocal(r_sb[:], z_sb[:])
                p_sb = sp.tile([8, 3, 3, C], F32R, tag="p")
                nc.vector.tensor_tensor(
                    p_sb[:], e_sb[:],
                    r_sb[:, :, None, :].to_broadcast((8, 3, 3, C)), ALU.mult)

                # ------------- weighted sum over j (via PE broadcast) -------------
                o_sb = ap.tile([128, 12, C], F32R, tag="o")
                for i in range(3):
                    for p in range(4):
                        pv = sp2.tile([128, 3, C], F32, tag="pv",
                                      name=f"pv{ch}_{i}_{p}")
                        for j in range(3):
                            pe_ps = psA.tile([128, C], F32, tag="mm",
                                             name=f"pe{ch}_{i}_{p}_{j}")
                            nc.tensor.matmul(
                                pe_ps[:], exps_sb[:, p, :],
                                p_sb[:, i, j, :], start=True, stop=True)
                            nc.vector.tensor_tensor(
                                pv[:, j, :], pe_ps[:], v_sb[:, j, p, :],
                                ALU.mult)
                        tmp = sp2.tile([128, C], F32, tag="jtmp",
                                       name=f"jt{ch}_{i}_{p}")
                        nc.vector.tensor_tensor(tmp[:], pv[:, 0, :],
                                                pv[:, 1, :], ALU.add)
                        nc.vector.tensor_tensor(o_sb[:, i * 4 + p, :], tmp[:],
                                                pv[:, 2, :], ALU.add)

                # ---------------- fused W1(out_proj .) + ReLU ----------------
                hpre_sb = ap.tile([128, 2, C], F32R, tag="hpre")
                for m2 in range(2):
                    hp = psA.tile([128, C], F32, tag="mm", name=f"hp{ch}_{m2}")
                    for kip in range(12):
                        nc.tensor.matmul(
                            hp[:], w1o_sb[:, kip, m2 * 128:(m2 + 1) * 128],
                            o_sb[:, kip, :],
                            start=(kip == 0), stop=(kip == 11))
                    nc.scalar.activation(hpre_sb[:, m2, :], hp[:], AF.Relu,
                                         bias=beff_sb[:, m2:m2 + 1], scale=1.0)

                # ---------------- fus2 ----------------
                y_sb = ap.tile([128, 4, C], F32R, tag="y")
                for m4 in range(4):
                    yp = psA.tile([128, C], F32, tag="mm", name=f"yp{ch}_{m4}")
                    for ks in range(2):
                        nc.tensor.matmul(
                            yp[:], w2_sb[:, ks, m4 * 128:(m4 + 1) * 128],
                            hpre_sb[:, ks, :], start=(ks == 0), stop=(ks == 1))
                    nc.scalar.activation(y_sb[:, m4, :], yp[:], AF.Identity,
                                         bias=b2_sb[:, m4:m4 + 1], scale=1.0)

                # ---------------- LayerNorm ----------------
                mu_ps = psT.tile([1, C], F32, tag="st", name=f"mu{ch}")
                for p in range(4):
                    nc.tensor.matmul(mu_ps[:], o512_sb[:], y_sb[:, p, :],
                                     start=(p == 0), stop=(p == 3))
                m2_ps = psT.tile([1, C], F32, tag="st", name=f"m2{ch}")
                for p in range(4):
                    ysq = sp2.tile([128, C], F32R, tag="ysq",
                                   name=f"ysq{ch}_{p}")
                    nc.scalar.activation(ysq[:], y_sb[:, p, :], AF.Square)
                    nc.tensor.matmul(m2_ps[:], o512_sb[:], ysq[:],
                                     start=(p == 0), stop=(p == 3))
                mu_sb = sp.tile([1, C], F32R, tag="musb")
                nc.scalar.copy(mu_sb[:], mu_ps[:])
                var_sb = sp.tile([1, C], F32, tag="varsb")
                # var = E[y^2] - mu^2  (psum m2 minus mu*mu)
                musq = sp.tile([1, C], F32, tag="musq")
                nc.vector.tensor_tensor(musq[:], mu_sb.bitcast(F32)[:],
                                        mu_sb.bitcast(F32)[:], ALU.mult)
                nc.vector.tensor_tensor(var_sb[:], m2_ps[:], musq[:],
                                        ALU.subtract)
                sd_sb = sp.tile([1, C], F32, tag="sdsb")
                nc.scalar.activation(sd_sb[:], var_sb[:], AF.Sqrt,
                                     bias=eps_sb[:], scale=1.0)
                rstd_sb = sp.tile([1, C], F32R, tag="rstdsb")
                with nc.allow_low_precision(reason="f32r is fp32 storage"):
                    nc.vector.reciprocal(rstd_sb[:], sd_sb[:])
                muex_ps = psT.tile([128, C], F32, tag="st", name=f"muex{ch}")
                nc.tensor.matmul(muex_ps[:], ok1_sb[:], mu_sb[:],
                                 start=True, stop=True)
                rsex_ps = psT.tile([128, C], F32, tag="st", name=f"rsex{ch}")
                nc.tensor.matmul(rsex_ps[:], ok1_sb[:], rstd_sb[:],
                                 start=True, stop=True)
                tdiff = ap.tile([128, 4, C], F32, tag="tdiff")
                nc.vector.tensor_tensor(
                    tdiff[:], y_sb.bitcast(F32)[:],
                    muex_ps[:, None, :].to_broadcast((128, 4, C)),
                    ALU.subtract)
                nc.vector.tensor_tensor(
                    tdiff[:], tdiff[:],
                    rsex_ps[:, None, :].to_broadcast((128, 4, C)), ALU.mult)
                fused = ap.tile([128, 4, C], F32R, tag="fused")
                for p in range(4):
                    nc.scalar.activation(fused[:, p, :], tdiff[:, p, :],
                                         AF.Identity,
                                         bias=lnb_sb[:, p:p + 1],
                                         scale=lng_sb[:, p:p + 1])

                # ---------------- routed expert matmul ----------------
                for m4 in range(4):
                    op = psA.tile([128, C], F32, tag="mm", name=f"op{ch}_{m4}")
                    for ks in range(4):
                        nc.tensor.matmul(
                            op[:],
                            waff_sb[:, ks,
                                    exp_idx * 512 + m4 * 128:
                                    exp_idx * 512 + (m4 + 1) * 128],
                            fused[:, ks, :], start=(ks == 0), stop=(ks == 3))
                    ot = sp2.tile([128, C], F32, tag="ot", name=f"ot{ch}_{m4}")
                    nc.scalar.activation(
                        ot[:], op[:], AF.Identity,
                        bias=baff_sb[:, exp_idx * 4 + m4:exp_idx * 4 + m4 + 1],
                        scale=1.0)
                    nc.sync.dma_start(outT[m4, :, col:col + C], ot[:])

    nc.finalize()
    return nc


def _prep_weights(inputs):
    in_proj_w = np.asarray(inputs["in_proj_w"], np.float32)
    in_proj_b = np.asarray(inputs["in_proj_b"], np.float32)
    out_proj_w = np.asarray(inputs["out_proj_w"], np.float32)
    out_proj_b = np.asarray(inputs["out_proj_b"], np.float32)
    fus_w1 = np.asarray(inputs["fus_w1"], np.float32)
    fus_b1 = np.asarray(inputs["fus_b1"], np.float32)
    fus_w2 = np.asarray(inputs["fus_w2"], np.float32)
    fus_b2 = np.asarray(inputs["fus_b2"], np.float32)
    ln_g = np.asarray(inputs["ln_g"], np.float32)
    ln_b = np.asarray(inputs["ln_b"], np.float32)
    aff_w = np.asarray(inputs["aff_w"], np.float32)
    aff_b = np.asarray(inputs["aff_b"], np.float32)

    scale = 1.0 / np.sqrt(np.float32(HD))
    W = in_proj_w.copy()
    W[:E] *= scale
    bq = in_proj_b.copy()
    bq[:E] *= scale
    # W.T is [512(k), 1536(m)]; sbuf wants [128, 4(ksub), 1536]
    wqkv_h = np.ascontiguousarray(
        W.T.reshape(4, 128, 1536).transpose(1, 0, 2))
    bqkv_h = np.ascontiguousarray(bq.reshape(12, 128).T)

    # fold out_proj into fus_w1; permute (h,d) -> (p, hl, d) to match v layout
    perm = np.empty(E, np.int64)
    for h in range(NH):
        for d in range(HD):
            perm[(h // 2) * 128 + (h % 2) * 64 + d] = h * HD + d
    blocks = []
    for i in range(3):
        blk = fus_w1[:, i * E:(i + 1) * E] @ out_proj_w  # [256, 512]
        blocks.append(blk[:, perm])
    W1o = np.concatenate(blocks, axis=1)  # [256, 1536] cols = (i, p, hl, d)
    w1o_h = np.ascontiguousarray(W1o.T.reshape(12, 128, 256).transpose(1, 0, 2))
    beff = fus_b1 + fus_w1 @ np.tile(out_proj_b, 3)
    beff_h = np.ascontiguousarray(beff.reshape(2, 128).T)

    w2_h = np.ascontiguousarray(fus_w2.T.reshape(2, 128, 512).transpose(1, 0, 2))
    b2_h = np.ascontiguousarray(fus_b2.reshape(4, 128).T)
    lng_h = np.ascontiguousarray(ln_g.reshape(4, 128).T)
    lnb_h = np.ascontiguousarray(ln_b.reshape(4, 128).T)

    A = np.concatenate([aff_w[e].T for e in range(NE)], axis=1)  # [512, 2048]
    waff_h = np.ascontiguousarray(A.reshape(4, 128, 2048).transpose(1, 0, 2))
    baff_h = np.ascontiguousarray(aff_b.reshape(NE * 4, 128).T)

    sel_h = np.zeros((128, 4, 8), np.float32)
    for r in range(128):
        for p in range(4):
            sel_h[r, p, 2 * p + r // 64] = 1.0
    exps_h = np.zeros((8, 4, 128), np.float32)
    for p in range(4):
        for c in range(128):
            exps_h[2 * p + c // 64, p, c] = 1.0

    bf = mybir.dt.np(BF16)
    return {
        "wqkv": wqkv_h, "bqkv": bqkv_h, "w1o": w1o_h, "beff": beff_h,
        "w2": w2_h, "b2": b2_h, "lng": lng_h, "lnb": lnb_h,
        "waff": waff_h, "baff": baff_h,
        "sel": sel_h.astype(bf), "exps": exps_h,
        "ones512": np.full((128, 1), 1.0 / E, np.float32),
        "onesk1": np.ones((1, 128), np.float32),
    }


def kernel(**inputs):
    img = np.asarray(inputs["image_embeddings"], np.float32)
    txt = np.asarray(inputs["text_embeddings"], np.float32)
    kno = np.asarray(inputs["knowledge_embeddings"], np.float32)
    labels = np.asarray(inputs["affective_labels"]).astype(np.int64).ravel()
    assert img.shape == (B, E)

    # ---- host-side expert routing (per-core fixed capacities) ----
    core_idx = np.zeros((NCORES, R), np.int64)
    core_val = np.zeros((NCORES, R), bool)
    for e in range(NE):
        ids = np.nonzero(labels == e)[0]
        assert len(ids) <= NCORES * CAP, f"expert {e} overflow: {len(ids)}"
        parts = np.array_split(ids, NCORES)
        for c in range(NCORES):
            seg = parts[c]
            core_idx[c, e * CAP: e * CAP + len(seg)] = seg
            core_val[c, e * CAP: e * CAP + len(seg)] = True

    wmap = _prep_weights(inputs)

    in_maps = []
    for c in range(NCORES):
        gi = core_idx[c]
        xg = np.stack([img[gi], txt[gi], kno[gi]])        # [3, R, 512]
        xg = xg.transpose(0, 2, 1)                        # [3, 512, R]
        xt_h = np.ascontiguousarray(xg.reshape(3, 4, 128, R))
        m = dict(wmap)
        m["xt"] = xt_h
        in_maps.append(m)

    nc = _build_program()
    res = run_bass_kernel_spmd(nc, in_maps, core_ids=list(range(NCORES)))
    global LAST_RESULTS, LAST_NC
    LAST_RESULTS = res
    LAST_NC = nc

    out_full = np.zeros((B, E), np.float32)
    for c in range(NCORES):
        oT = res.results[c]["outT"].reshape(E, R).T       # [R, 512]
        v = core_val[c]
        out_full[core_idx[c][v]] = oT[v]
    return out_full


if __name__ == "__main__":
    rng = np.random.default_rng(0)
    fake = {
        "image_embeddings": rng.standard_normal((B, E)).astype(np.float32),
        "text_embeddings": rng.standard_normal((B, E)).astype(np.float32),
        "knowledge_embeddings": rng.standard_normal((B, E)).astype(np.float32),
        "affective_labels": rng.integers(0, NE, B),
        "in_proj_w": (rng.standard_normal((3 * E, E)) * 0.02).astype(np.float32),
        "in_proj_b": np.zeros(3 * E, np.float32),
        "out_proj_w": (rng.standard_normal((E, E)) * 0.02).astype(np.float32),
        "out_proj_b": np.zeros(E, np.float32),
        "fus_w1": (rng.standard_normal((H, 3 * E)) * 0.02).astype(np.float32),
        "fus_b1": np.zeros(H, np.float32),
        "fus_w2": (rng.standard_normal((E, H)) * 0.02).astype(np.float32),
        "fus_b2": np.zeros(E, np.float32),
        "ln_g": np.ones(E, np.float32),
        "ln_b": np.zeros(E, np.float32),
        "aff_w": (rng.standard_normal((NE, E, E)) * 0.02).astype(np.float32),
        "aff_b": np.zeros((NE, E), np.float32),
    }
    out = kernel(**fake)
    print("kernel ran, out:", out.shape, out.dtype, np.abs(out).max())


# revision 35
# speedup vs baseline: 1.0058x; 1.0058x over previous
"""Trainium2 Bass kernel for nn_MultiModalFusion (moe_routing).

Strategy:
- Pure data-parallel over 8 cores; host sorts samples by expert label so each
  core sees 4 contiguous expert groups of fixed capacity (static shapes, only
  1 of 4 expert matmuls runs per sample).
- Feature-partitioned ("transposed") layout on device: activations are
  [feature, sample]; all dense math is weight-stationary fp32r matmuls
  (1 cyc/row for N>=256, ~1.5e-4 rel err).
- out_proj is folded into fus_w1 on the host (saves a full [B*3,512]x[512,512]
  matmul); the 1/sqrt(hd) score scale is folded into W_q.
- Tiny-seq (3 tokens) attention: q*k products on DVE (bf16, 2x mode);
  per-head d-reduction and softmax probability broadcast-over-d are done with
  small constant selection-matrix matmuls on the tensor engine.
"""

import numpy as np

import concourse.bass as bass
import concourse.mybir as mybir
import concourse.tile as tile
from concourse import bacc
from concourse.bass_utils import run_bass_kernel_spmd

E = 512
H = 256
NH = 8
HD = 64
NE = 4
B = 16384
NCORES = 8
CAP = 544            # per-core per-expert capacity (ceil(B/4/8) + slack)
R = NE * CAP         # 2176 columns per core
C = 272              # chunk = half an expert group; 8 chunks, all N>=256
NCH = R // C

LAST_RESULTS = None  # BassKernelResults of the most recent kernel() call
LAST_NC = None       # finalized Bass program of the most recent kernel() call

F32 = mybir.dt.float32
F32R = mybir.dt.float32r
BF16 = mybir.dt.bfloat16
AF = mybir.ActivationFunctionType
ALU = mybir.AluOpType


def _build_program():
    nc = bacc.Bacc("TRN2")

    # ---------------- DRAM I/O ----------------
    xt = nc.dram_tensor("xt", [3, 4, 128, R], F32R, kind="ExternalInput")
    wqkv = nc.dram_tensor("wqkv", [128, 4, 1536], F32R, kind="ExternalInput")
    bqkv = nc.dram_tensor("bqkv", [128, 12], F32, kind="ExternalInput")
    w1o = nc.dram_tensor("w1o", [128, 12, 256], F32R, kind="ExternalInput")
    beff = nc.dram_tensor("beff", [128, 2], F32, kind="ExternalInput")
    w2 = nc.dram_tensor("w2", [128, 2, 512], F32R, kind="ExternalInput")
    b2 = nc.dram_tensor("b2", [128, 4], F32, kind="ExternalInput")
    lng = nc.dram_tensor("lng", [128, 4], F32, kind="ExternalInput")
    lnb = nc.dram_tensor("lnb", [128, 4], F32, kind="ExternalInput")
    waff = nc.dram_tensor("waff", [128, 4, 2048], F32R, kind="ExternalInput")
    baff = nc.dram_tensor("baff", [128, 16], F32, kind="ExternalInput")
    sel = nc.dram_tensor("sel", [128, 4, 8], BF16, kind="ExternalInput")
    exps = nc.dram_tensor("exps", [8, 4, 128], F32R, kind="ExternalInput")
    ones512 = nc.dram_tensor("ones512", [128, 1], F32R, kind="ExternalInput")
    onesk1 = nc.dram_tensor("onesk1", [1, 128], F32R, kind="ExternalInput")
    outT = nc.dram_tensor("outT", [4, 128, R], F32, kind="ExternalOutput")

    with tile.TileContext(nc) as tc:
        with tc.tile_pool(name="wp", bufs=1) as wp, \
             tc.tile_pool(name="xp", bufs=2) as xp, \
             tc.tile_pool(name="ap", bufs=1) as ap, \
             tc.tile_pool(name="ap2", bufs=2) as ap2, \
             tc.tile_pool(name="sp", bufs=1) as sp, \
             tc.tile_pool(name="sp2", bufs=2) as sp2, \
             tc.tile_pool(name="psA", bufs=4, space="PSUM") as psA, \
             tc.tile_pool(name="psS", bufs=2, space="PSUM") as psS, \
             tc.tile_pool(name="psT", bufs=2, space="PSUM") as psT:

            # ---------------- load weights/constants once ----------------
            wqkv_sb = wp.tile([128, 4, 1536], F32R)
            nc.sync.dma_start(wqkv_sb[:], wqkv[:])
            bqkv_sb = wp.tile([128, 12], F32)
            nc.sync.dma_start(bqkv_sb[:], bqkv[:])
            w1o_sb = wp.tile([128, 12, 256], F32R)
            nc.sync.dma_start(w1o_sb[:], w1o[:])
            beff_sb = wp.tile([128, 2], F32)
            nc.sync.dma_start(beff_sb[:], beff[:])
            w2_sb = wp.tile([128, 2, 512], F32R)
            nc.sync.dma_start(w2_sb[:], w2[:])
            b2_sb = wp.tile([128, 4], F32)
            nc.sync.dma_start(b2_sb[:], b2[:])
            lng_sb = wp.tile([128, 4], F32)
            nc.sync.dma_start(lng_sb[:], lng[:])
            lnb_sb = wp.tile([128, 4], F32)
            nc.sync.dma_start(lnb_sb[:], lnb[:])

            baff_sb = wp.tile([128, 16], F32)
            nc.sync.dma_start(baff_sb[:], baff[:])
            sel_sb = wp.tile([128, 4, 8], BF16)
            nc.sync.dma_start(sel_sb[:], sel[:])
            exps_sb = wp.tile([8, 4, 128], F32R)
            nc.sync.dma_start(exps_sb[:], exps[:])
            o512_sb = wp.tile([128, 1], F32R)
            nc.sync.dma_start(o512_sb[:], ones512[:])
            ok1_sb = wp.tile([1, 128], F32R)
            nc.sync.dma_start(ok1_sb[:], onesk1[:])
            eps_sb = wp.tile([1, 1], F32)
            nc.vector.memset(eps_sb[:], 1e-5)

            for ch in range(NCH):
                col = ch * C
                exp_idx = ch // 2  # expert for this chunk

                # ---------------- load x chunk ----------------
                x_sb = xp.tile([128, 3, 4, C], F32R, tag="x")
                for t in range(3):
                    for p in range(4):
                        nc.sync.dma_start(x_sb[:, t, p, :],
                                          xt[t, p, :, col:col + C])

                # ---------------- QKV projection ----------------
                q_sb = ap.tile([128, 3, 4, C], BF16, tag="q")
                k_sb = ap.tile([128, 3, 4, C], BF16, tag="k")
                v_sb = ap2.tile([128, 3, 4, C], F32R, tag="v")
                waff_sb = ap2.tile([128, 4, 512], F32R, tag="waff")
                nc.sync.dma_start(
                    waff_sb[:],
                    waff[:, :, exp_idx * 512:(exp_idx + 1) * 512])
                for t in range(3):
                    for mi in range(12):
                        qp = psA.tile([128, C], F32, tag="mm", name=f"qkv{ch}_{t}_{mi}")
                        for ks in range(4):
                            nc.tensor.matmul(
                                qp[:],
                                wqkv_sb[:, ks, mi * 128:(mi + 1) * 128],
                                x_sb[:, t, ks, :],
                                start=(ks == 0), stop=(ks == 3))
                        dst = (q_sb, k_sb, v_sb)[mi // 4]
                        nc.scalar.activation(
                            dst[:, t, mi % 4, :], qp[:], AF.Identity,
                            bias=bqkv_sb[:, mi:mi + 1], scale=1.0)

                # ---------------- attention scores ----------------
                e_sb = sp.tile([8, 3, 3, C], F32R, tag="esb")
                for i in range(3):
                    for j in range(3):
                        prod = sp2.tile([128, 4, C], BF16, tag="prod",
                                        name=f"prod{ch}_{i}_{j}")
                        nc.vector.tensor_tensor(
                            prod[:], q_sb[:, i, :, :], k_sb[:, j, :, :],
                            ALU.mult)
                        s_ps = psS.tile([8, C], F32, tag="s",
                                        name=f"s{ch}_{i}_{j}")
                        for p in range(4):
                            nc.tensor.matmul(
                                s_ps[:], sel_sb[:, p, :], prod[:, p, :],
                                start=(p == 0), stop=(p == 3))
                        nc.scalar.activation(
                            e_sb[:, i, j, :], s_ps[:], AF.Exp)

                # ---------------- softmax over j ----------------
                e_f32 = e_sb.bitcast(F32)
                z_sb = sp.tile([8, 3, C], F32, tag="z")
                nc.vector.tensor_tensor(z_sb[:], e_f32[:, :, 0, :],
                                        e_f32[:, :, 1, :], ALU.add)
                nc.vector.tensor_tensor(z_sb[:], z_sb[:], e_f32[:, :, 2, :],
                                        ALU.add)
                nc.vector.reciprocal_approx_fast(z_sb[:], z_sb[:])
                p_sb = e_sb
                nc.vector.tensor_tensor(
                    p_sb[:], e_f32[:],
                    z_sb[:, :, None, :].to_broadcast((8, 3, 3, C)), ALU.mult)

                # ------------- weighted sum over j (via PE broadcast) -------------
                o_sb = ap.tile([128, 12, C], F32R, tag="o")
                for i in range(3):
                    for p in range(4):
                        pv = sp2.tile([128, 3, C], F32, tag="pv",
                                      name=f"pv{ch}_{i}_{p}")
                        for j in range(3):
                            pe_ps = psA.tile([128, C], F32, tag="mm",
                                             name=f"pe{ch}_{i}_{p}_{j}")
                            nc.tensor.matmul(
                                pe_ps[:], exps_sb[:, p, :],
                                p_sb[:, i, j, :], start=True, stop=True)
                            nc.vector.tensor_tensor(
                                pv[:, j, :], pe_ps[:], v_sb[:, j, p, :],
                                ALU.mult)
                        tmp = sp2.tile([128, C], F32, tag="jtmp",
                                       name=f"jt{ch}_{i}_{p}")
                        nc.vector.tensor_tensor(tmp[:], pv[:, 0, :],
                                                pv[:, 1, :], ALU.add)
                        nc.vector.tensor_tensor(o_sb[:, i * 4 + p, :], tmp[:],
                                                pv[:, 2, :], ALU.add)

                # ---------------- fused W1(out_proj .) + ReLU ----------------
                hpre_sb = ap.tile([128, 2, C], F32R, tag="hpre")
                for m2 in range(2):
                    hp = psA.tile([128, C], F32, tag="mm", name=f"hp{ch}_{m2}")
                    for kip in range(12):
                        nc.tensor.matmul(
                            hp[:], w1o_sb[:, kip, m2 * 128:(m2 + 1) * 128],
                            o_sb[:, kip, :],
                            start=(kip == 0), stop=(kip == 11))
                    nc.scalar.activation(hpre_sb[:, m2, :], hp[:], AF.Relu,
                                         bias=beff_sb[:, m2:m2 + 1], scale=1.0)

                # ---------------- fus2 ----------------
                y_sb = ap.tile([128, 4, C], F32R, tag="y")
                for m4 in range(4):
                    yp = psA.tile([128, C], F32, tag="mm", name=f"yp{ch}_{m4}")
                    for ks in range(2):
                        nc.tensor.matmul(
                            yp[:], w2_sb[:, ks, m4 * 128:(m4 + 1) * 128],
                            hpre_sb[:, ks, :], start=(ks == 0), stop=(ks == 1))
                    nc.scalar.activation(y_sb[:, m4, :], yp[:], AF.Identity,
                                         bias=b2_sb[:, m4:m4 + 1], scale=1.0)

                # ---------------- LayerNorm ----------------
                mu_ps = psT.tile([1, C], F32, tag="st", name=f"mu{ch}")
                for p in range(4):
                    nc.tensor.matmul(mu_ps[:], o512_sb[:], y_sb[:, p, :],
                                     start=(p == 0), stop=(p == 3))
                m2_ps = psT.tile([1, C], F32, tag="st", name=f"m2{ch}")
                for p in range(4):
                    ysq = sp2.tile([128, C], F32R, tag="ysq",
                                   name=f"ysq{ch}_{p}")
                    nc.scalar.activation(ysq[:], y_sb[:, p, :], AF.Square)
                    nc.tensor.matmul(m2_ps[:], o512_sb[:], ysq[:],
                                     start=(p == 0), stop=(p == 3))
                mu_sb = sp.tile([1, C], F32R, tag="musb")
                nc.scalar.copy(mu_sb[:], mu_ps[:])
                var_sb = sp.tile([1, C], F32, tag="varsb")
                # var = E[y^2] - mu^2  (psum m2 minus mu*mu)
                musq = sp.tile([1, C], F32, tag="musq")
                nc.vector.tensor_tensor(musq[:], mu_sb.bitcast(F32)[:],
                                        mu_sb.bitcast(F32)[:], ALU.mult)
                nc.vector.tensor_tensor(var_sb[:], m2_ps[:], musq[:],
                                        ALU.subtract)
                sd_sb = sp.tile([1, C], F32, tag="sdsb")
                nc.scalar.activation(sd_sb[:], var_sb[:], AF.Sqrt,
                                     bias=eps_sb[:], scale=1.0)
                rstd_f = sp.tile([1, C], F32, tag="rstdf")
                nc.vector.reciprocal_approx_fast(rstd_f[:], sd_sb[:])
                rstd_sb = sp.tile([1, C], F32R, tag="rstdsb")
                nc.scalar.copy(rstd_sb[:], rstd_f[:])
                muex_ps = psT.tile([128, C], F32, tag="st", name=f"muex{ch}")
                nc.tensor.matmul(muex_ps[:], ok1_sb[:], mu_sb[:],
                                 start=True, stop=True)
                rsex_ps = psT.tile([128, C], F32, tag="st", name=f"rsex{ch}")
                nc.tensor.matmul(rsex_ps[:], ok1_sb[:], rstd_sb[:],
                                 start=True, stop=True)
                fused = ap.tile([128, 4, C], F32R, tag="fused")
                for p in range(4):
                    lnp = sp2.tile([128, C], F32, tag="lnp",
                                   name=f"lnp{ch}_{p}")
                    nc.vector.tensor_tensor(lnp[:], y_sb.bitcast(F32)[:, p, :],
                                            muex_ps[:], ALU.subtract)
                    nc.vector.tensor_tensor(lnp[:], lnp[:], rsex_ps[:],
                                            ALU.mult)
                    nc.scalar.activation(fused[:, p, :], lnp[:],
                                         AF.Identity,
                                         bias=lnb_sb[:, p:p + 1],
                                         scale=lng_sb[:, p:p + 1])

                # ---------------- routed expert matmul ----------------
                for m4 in range(4):
                    op = psA.tile([128, C], F32, tag="mm", name=f"op{ch}_{m4}")
                    for ks in range(4):
                        nc.tensor.matmul(
                            op[:],
                            waff_sb[:, ks, m4 * 128:(m4 + 1) * 128],
                            fused[:, ks, :], start=(ks == 0), stop=(ks == 3))
                    ot = sp2.tile([128, C], F32, tag="ot", name=f"ot{ch}_{m4}")
                    nc.scalar.activation(
                        ot[:], op[:], AF.Identity,
                        bias=baff_sb[:, exp_idx * 4 + m4:exp_idx * 4 + m4 + 1],
                        scale=1.0)
                    nc.sync.dma_start(outT[m4, :, col:col + C], ot[:])

    nc.finalize()
    return nc


def _prep_weights(inputs):
    in_proj_w = np.asarray(inputs["in_proj_w"], np.float32)
    in_proj_b = np.asarray(inputs["in_proj_b"], np.float32)
    out_proj_w = np.asarray(inputs["out_proj_w"], np.float32)
    out_proj_b = np.asarray(inputs["out_proj_b"], np.float32)
    fus_w1 = np.asarray(inputs["fus_w1"], np.float32)
    fus_b1 = np.asarray(inputs["fus_b1"], np.float32)
    fus_w2 = np.asarray(inputs["fus_w2"], np.float32)
    fus_b2 = np.asarray(inputs["fus_b2"], np.float32)
    ln_g = np.asarray(inputs["ln_g"], np.float32)
    ln_b = np.asarray(inputs["ln_b"], np.float32)
    aff_w = np.asarray(inputs["aff_w"], np.float32)
    aff_b = np.asarray(inputs["aff_b"], np.float32)

    scale = 1.0 / np.sqrt(np.float32(HD))
    W = in_proj_w.copy()
    W[:E] *= scale
    bq = in_proj_b.copy()
    bq[:E] *= scale
    # W.T is [512(k), 1536(m)]; sbuf wants [128, 4(ksub), 1536]
    wqkv_h = np.ascontiguousarray(
        W.T.reshape(4, 128, 1536).transpose(1, 0, 2))
    bqkv_h = np.ascontiguousarray(bq.reshape(12, 128).T)

    # fold out_proj into fus_w1; permute (h,d) -> (p, hl, d) to match v layout
    perm = np.empty(E, np.int64)
    for h in range(NH):
        for d in range(HD):
            perm[(h // 2) * 128 + (h % 2) * 64 + d] = h * HD + d
    blocks = []
    for i in range(3):
        blk = fus_w1[:, i * E:(i + 1) * E] @ out_proj_w  # [256, 512]
        blocks.append(blk[:, perm])
    W1o = np.concatenate(blocks, axis=1)  # [256, 1536] cols = (i, p, hl, d)
    w1o_h = np.ascontiguousarray(W1o.T.reshape(12, 128, 256).transpose(1, 0, 2))
    beff = fus_b1 + fus_w1 @ np.tile(out_proj_b, 3)
    beff_h = np.ascontiguousarray(beff.reshape(2, 128).T)

    w2_h = np.ascontiguousarray(fus_w2.T.reshape(2, 128, 512).transpose(1, 0, 2))
    b2_h = np.ascontiguousarray(fus_b2.reshape(4, 128).T)
    lng_h = np.ascontiguousarray(ln_g.reshape(4, 128).T)
    lnb_h = np.ascontiguousarray(ln_b.reshape(4, 128).T)

    A = np.concatenate([aff_w[e].T for e in range(NE)], axis=1)  # [512, 2048]
    waff_h = np.ascontiguousarray(A.reshape(4, 128, 2048).transpose(1, 0, 2))
    baff_h = np.ascontiguousarray(aff_b.reshape(NE * 4, 128).T)

    sel_h = np.zeros((128, 4, 8), np.float32)
    for r in range(128):
        for p in range(4):
            sel_h[r, p, 2 * p + r // 64] = 1.0
    exps_h = np.zeros((8, 4, 128), np.float32)
    for p in range(4):
        for c in range(128):
            exps_h[2 * p + c // 64, p, c] = 1.0

    bf = mybir.dt.np(BF16)
    return {
        "wqkv": wqkv_h, "bqkv": bqkv_h, "w1o": w1o_h, "beff": beff_h,
        "w2": w2_h, "b2": b2_h, "lng": lng_h, "lnb": lnb_h,
        "waff": waff_h, "baff": baff_h,
        "sel": sel_h.astype(bf), "exps": exps_h,
        "ones512": np.full((128, 1), 1.0 / E, np.float32),
        "onesk1": np.ones((1, 128), np.float32),
    }


def kernel(**inputs):
    img = np.asarray(inputs["image_embeddings"], np.float32)
    txt = np.asarray(inputs["text_embeddings"], np.float32)
    kno = np.asarray(inputs["knowledge_embeddings"], np.float32)
    labels = np.asarray(inputs["affective_labels"]).astype(np.int64).ravel()
    assert img.shape == (B, E)

    # ---- host-side expert routing (per-core fixed capacities) ----
    core_idx = np.zeros((NCORES, R), np.int64)
    core_val = np.zeros((NCORES, R), bool)
    for e in range(NE):
        ids = np.nonzero(labels == e)[0]
        assert len(ids) <= NCORES * CAP, f"expert {e} overflow: {len(ids)}"
        parts = np.array_split(ids, NCORES)
        for c in range(NCORES):
            seg = parts[c]
            core_idx[c, e * CAP: e * CAP + len(seg)] = seg
            core_val[c, e * CAP: e * CAP + len(seg)] = True

    wmap = _prep_weights(inputs)

    in_maps = []
    for c in range(NCORES):
        gi = core_idx[c]
        xg = np.stack([img[gi], txt[gi], kno[gi]])        # [3, R, 512]
        xg = xg.transpose(0, 2, 1)                        # [3, 512, R]
        xt_h = np.ascontiguousarray(xg.reshape(3, 4, 128, R))
        m = dict(wmap)
        m["xt"] = xt_h
        in_maps.append(m)

    nc = _build_program()
    res = run_bass_kernel_spmd(nc, in_maps, core_ids=list(range(NCORES)))
    global LAST_RESULTS, LAST_NC
    LAST_RESULTS = res
    LAST_NC = nc

    out_full = np.zeros((B, E), np.float32)
    for c in range(NCORES):
        oT = res.results[c]["outT"].reshape(E, R).T       # [R, 512]
        v = core_val[c]
        out_full[core_idx[c][v]] = oT[v]
    return out_full


if __name__ == "__main__":
    rng = np.random.default_rng(0)
    fake = {
        "image_embeddings": rng.standard_normal((B, E)).astype(np.float32),
        "text_embeddings": rng.standard_normal((B, E)).astype(np.float32),
        "knowledge_embeddings": rng.standard_normal((B, E)).astype(np.float32),
        "affective_labels": rng.integers(0, NE, B),
        "in_proj_w": (rng.standard_normal((3 * E, E)) * 0.02).astype(np.float32),
        "in_proj_b": np.zeros(3 * E, np.float32),
        "out_proj_w": (rng.standard_normal((E, E)) * 0.02).astype(np.float32),
        "out_proj_b": np.zeros(E, np.float32),
        "fus_w1": (rng.standard_normal((H, 3 * E)) * 0.02).astype(np.float32),
        "fus_b1": np.zeros(H, np.float32),
        "fus_w2": (rng.standard_normal((E, H)) * 0.02).astype(np.float32),
        "fus_b2": np.zeros(E, np.float32),
        "ln_g": np.ones(E, np.float32),
        "ln_b": np.zeros(E, np.float32),
        "aff_w": (rng.standard_normal((NE, E, E)) * 0.02).astype(np.float32),
        "aff_b": np.zeros((NE, E), np.float32),
    }
    out = kernel(**fake)
    print("kernel ran, out:", out.shape, out.dtype, np.abs(out).max())


# revision 42
# speedup vs baseline: 1.3681x; 1.3602x over previous
"""Trainium2 Bass kernel for nn_MultiModalFusion (moe_routing).

Strategy:
- Pure data-parallel over 8 cores; host sorts samples by expert label so each
  core sees 4 contiguous expert groups of fixed capacity (static shapes, only
  1 of 4 expert matmuls runs per sample).
- Feature-partitioned ("transposed") layout on device: activations are
  [feature, sample]; all dense math is weight-stationary fp32r matmuls
  (1 cyc/row for N>=256, ~1.5e-4 rel err).
- out_proj is folded into fus_w1 on the host (saves a full [B*3,512]x[512,512]
  matmul); the 1/sqrt(hd) score scale is folded into W_q.
- Tiny-seq (3 tokens) attention: q*k products on DVE (bf16, 2x mode);
  per-head d-reduction and softmax probability broadcast-over-d are done with
  small constant selection-matrix matmuls on the tensor engine.
"""

import numpy as np

import concourse.bass as bass
import concourse.mybir as mybir
import concourse.tile as tile
from concourse import bacc
from concourse.bass_utils import run_bass_kernel_spmd

E = 512
H = 256
NH = 8
HD = 64
NE = 4
B = 16384
NCORES = 8
CAP = 544            # per-core per-expert capacity (ceil(B/4/8) + slack)
R = NE * CAP         # 2176 columns per core
C = 272              # chunk = half an expert group; 8 chunks, all N>=256
NCH = R // C

LAST_RESULTS = None  # BassKernelResults of the most recent kernel() call
LAST_NC = None       # finalized Bass program of the most recent kernel() call

F32 = mybir.dt.float32
F32R = mybir.dt.float32r
BF16 = mybir.dt.bfloat16
AF = mybir.ActivationFunctionType
ALU = mybir.AluOpType


def _build_program():
    nc = bacc.Bacc("TRN2")

    # ---------------- DRAM I/O ----------------
    xt = nc.dram_tensor("xt", [3, 4, 128, R], F32R, kind="ExternalInput")
    wqkv = nc.dram_tensor("wqkv", [128, 4, 1536], F32R, kind="ExternalInput")
    bqkv = nc.dram_tensor("bqkv", [128, 12], F32, kind="ExternalInput")
    w1o = nc.dram_tensor("w1o", [128, 12, 256], F32R, kind="ExternalInput")
    beff = nc.dram_tensor("beff", [128, 2], F32, kind="ExternalInput")
    w2 = nc.dram_tensor("w2", [128, 2, 512], F32R, kind="ExternalInput")
    b2 = nc.dram_tensor("b2", [128, 4], F32, kind="ExternalInput")
    lng = nc.dram_tensor("lng", [128, 4], F32, kind="ExternalInput")
    lnb = nc.dram_tensor("lnb", [128, 4], F32, kind="ExternalInput")
    waff = nc.dram_tensor("waff", [128, 4, 2048], F32R, kind="ExternalInput")
    baff = nc.dram_tensor("baff", [128, 16], F32, kind="ExternalInput")
    sel = nc.dram_tensor("sel", [128, 4, 8], BF16, kind="ExternalInput")
    exps = nc.dram_tensor("exps", [8, 4, 128], F32R, kind="ExternalInput")
    ones512 = nc.dram_tensor("ones512", [128, 1], F32R, kind="ExternalInput")
    onesk1 = nc.dram_tensor("onesk1", [1, 128], F32R, kind="ExternalInput")
    outT = nc.dram_tensor("outT", [4, 128, R], F32, kind="ExternalOutput")

    with tile.TileContext(nc) as tc:
        with tc.tile_pool(name="wp", bufs=1) as wp, \
             tc.tile_pool(name="xp", bufs=2) as xp, \
             tc.tile_pool(name="ap", bufs=1) as ap, \
             tc.tile_pool(name="ap2", bufs=2) as ap2, \
             tc.tile_pool(name="sp", bufs=1) as sp, \
             tc.tile_pool(name="sp2", bufs=2) as sp2, \
             tc.tile_pool(name="psQ", bufs=2, space="PSUM") as psQ, \
             tc.tile_pool(name="psE", bufs=2, space="PSUM") as psE, \
             tc.tile_pool(name="psH", bufs=1, space="PSUM") as psH, \
             tc.tile_pool(name="psS", bufs=1, space="PSUM") as psS, \
             tc.tile_pool(name="psT", bufs=2, space="PSUM") as psT:

            # ---------------- load weights/constants once ----------------
            wqkv_sb = wp.tile([128, 4, 1536], F32R)
            nc.sync.dma_start(wqkv_sb[:], wqkv[:])
            bqkv_sb = wp.tile([128, 12], F32)
            nc.sync.dma_start(bqkv_sb[:], bqkv[:])
            w1o_sb = wp.tile([128, 12, 256], F32R)
            nc.sync.dma_start(w1o_sb[:], w1o[:])
            beff_sb = wp.tile([128, 2], F32)
            nc.sync.dma_start(beff_sb[:], beff[:])
            w2_sb = wp.tile([128, 2, 512], F32R)
            nc.sync.dma_start(w2_sb[:], w2[:])
            b2_sb = wp.tile([128, 4], F32)
            nc.sync.dma_start(b2_sb[:], b2[:])
            lng_sb = wp.tile([128, 4], F32)
            nc.sync.dma_start(lng_sb[:], lng[:])
            lnb_sb = wp.tile([128, 4], F32)
            nc.sync.dma_start(lnb_sb[:], lnb[:])

            baff_sb = wp.tile([128, 16], F32)
            nc.sync.dma_start(baff_sb[:], baff[:])
            sel_sb = wp.tile([128, 4, 8], BF16)
            nc.sync.dma_start(sel_sb[:], sel[:])
            exps_sb = wp.tile([8, 4, 128], F32R)
            nc.sync.dma_start(exps_sb[:], exps[:])
            o512_sb = wp.tile([128, 1], F32R)
            nc.sync.dma_start(o512_sb[:], ones512[:])
            ok1_sb = wp.tile([1, 128], F32R)
            nc.sync.dma_start(ok1_sb[:], onesk1[:])
            eps_sb = wp.tile([1, 1], F32)
            nc.vector.memset(eps_sb[:], 1e-5)

            for ch in range(NCH):
                col = ch * C
                exp_idx = ch // 2  # expert for this chunk

                # ---------------- load x chunk ----------------
                x_sb = xp.tile([128, 3, 4, C], F32R, tag="x")
                for t in range(3):
                    for p in range(4):
                        nc.sync.dma_start(x_sb[:, t, p, :],
                                          xt[t, p, :, col:col + C])

                # ---------------- QKV projection ----------------
                q_sb = ap.tile([128, 3, 4, C], BF16, tag="q")
                k_sb = ap.tile([128, 3, 4, C], BF16, tag="k")
                v_sb = ap2.tile([128, 3, 4, C], F32R, tag="v")
                waff_sb = ap2.tile([128, 4, 512], F32R, tag="waff")
                nc.sync.dma_start(
                    waff_sb[:],
                    waff[:, :, exp_idx * 512:(exp_idx + 1) * 512])
                for t in range(3):
                    for mi in range(12):
                        qp = psQ.tile([128, C], F32, tag="qkv", name=f"qkv{ch}_{t}_{mi}")
                        for ks in range(4):
                            nc.tensor.matmul(
                                qp[:],
                                wqkv_sb[:, ks, mi * 128:(mi + 1) * 128],
                                x_sb[:, t, ks, :],
                                start=(ks == 0), stop=(ks == 3))
                        dst = (q_sb, k_sb, v_sb)[mi // 4]
                        nc.scalar.activation(
                            dst[:, t, mi % 4, :], qp[:], AF.Identity,
                            bias=bqkv_sb[:, mi:mi + 1], scale=1.0)

                # ---------------- attention scores ----------------
                e_sb = sp.tile([8, 3, 3, C], F32R, tag="esb")
                for i in range(3):
                    for j in range(3):
                        prod = sp2.tile([128, 4, C], BF16, tag="prod",
                                        name=f"prod{ch}_{i}_{j}")
                        nc.vector.tensor_tensor(
                            prod[:], q_sb[:, i, :, :], k_sb[:, j, :, :],
                            ALU.mult)
                        s_ps = psS.tile([8, C], F32, tag="s",
                                        name=f"s{ch}_{i}_{j}")
                        for p in range(4):
                            nc.tensor.matmul(
                                s_ps[:], sel_sb[:, p, :], prod[:, p, :],
                                start=(p == 0), stop=(p == 3))
                        nc.scalar.activation(
                            e_sb[:, i, j, :], s_ps[:], AF.Exp)

                # ---------------- softmax over j ----------------
                e_f32 = e_sb.bitcast(F32)
                z_sb = sp.tile([8, 3, C], F32, tag="z")
                nc.vector.tensor_tensor(z_sb[:], e_f32[:, :, 0, :],
                                        e_f32[:, :, 1, :], ALU.add)
                nc.vector.tensor_tensor(z_sb[:], z_sb[:], e_f32[:, :, 2, :],
                                        ALU.add)
                nc.vector.reciprocal_approx_fast(z_sb[:], z_sb[:])
                p_sb = e_sb
                nc.vector.tensor_tensor(
                    p_sb[:], e_f32[:],
                    z_sb[:, :, None, :].to_broadcast((8, 3, 3, C)), ALU.mult)

                # ------------- weighted sum over j (via PE broadcast) -------------
                o_sb = ap.tile([128, 12, C], F32R, tag="o")
                for i in range(3):
                    for p in range(4):
                        pv = sp2.tile([128, 3, C], F32, tag="pv",
                                      name=f"pv{ch}_{i}_{p}")
                        for j in range(3):
                            pe_ps = psE.tile([128, C], F32, tag="pexp",
                                             name=f"pe{ch}_{i}_{p}_{j}")
                            nc.tensor.matmul(
                                pe_ps[:], exps_sb[:, p, :],
                                p_sb[:, i, j, :], start=True, stop=True)
                            nc.vector.tensor_tensor(
                                pv[:, j, :], pe_ps[:], v_sb[:, j, p, :],
                                ALU.mult)
                        tmp = sp2.tile([128, C], F32, tag="jtmp",
                                       name=f"jt{ch}_{i}_{p}")
                        nc.vector.tensor_tensor(tmp[:], pv[:, 0, :],
                                                pv[:, 1, :], ALU.add)
                        nc.vector.tensor_tensor(o_sb[:, i * 4 + p, :], tmp[:],
                                                pv[:, 2, :], ALU.add)

                # ---------------- fused W1(out_proj .) + ReLU ----------------
                hpre_sb = ap.tile([128, 2, C], F32R, tag="hpre")
                for m2 in range(2):
                    hp = psH.tile([128, C], F32, tag="tail", name=f"hp{ch}_{m2}")
                    for kip in range(12):
                        nc.tensor.matmul(
                            hp[:], w1o_sb[:, kip, m2 * 128:(m2 + 1) * 128],
                            o_sb[:, kip, :],
                            start=(kip == 0), stop=(kip == 11))
                    nc.scalar.activation(hpre_sb[:, m2, :], hp[:], AF.Relu,
                                         bias=beff_sb[:, m2:m2 + 1], scale=1.0)

                # ---------------- fus2 ----------------
                y_sb = ap.tile([128, 4, C], F32R, tag="y")
                for m4 in range(4):
                    yp = psH.tile([128, C], F32, tag="tail", name=f"yp{ch}_{m4}")
                    for ks in range(2):
                        nc.tensor.matmul(
                            yp[:], w2_sb[:, ks, m4 * 128:(m4 + 1) * 128],
                            hpre_sb[:, ks, :], start=(ks == 0), stop=(ks == 1))
                    nc.scalar.activation(y_sb[:, m4, :], yp[:], AF.Identity,
                                         bias=b2_sb[:, m4:m4 + 1], scale=1.0)

                # ---------------- LayerNorm ----------------
                mu_ps = psT.tile([1, C], F32, tag="st", name=f"mu{ch}")
                for p in range(4):
                    nc.tensor.matmul(mu_ps[:], o512_sb[:], y_sb[:, p, :],
                                     start=(p == 0), stop=(p == 3))
                m2_ps = psT.tile([1, C], F32, tag="st", name=f"m2{ch}")
                for p in range(4):
                    ysq = sp2.tile([128, C], F32R, tag="ysq",
                                   name=f"ysq{ch}_{p}")
                    nc.scalar.activation(ysq[:], y_sb[:, p, :], AF.Square)
                    nc.tensor.matmul(m2_ps[:], o512_sb[:], ysq[:],
                                     start=(p == 0), stop=(p == 3))
                mu_sb = sp.tile([1, C], F32R, tag="musb")
                nc.scalar.copy(mu_sb[:], mu_ps[:])
                var_sb = sp.tile([1, C], F32, tag="varsb")
                # var = E[y^2] - mu^2  (psum m2 minus mu*mu)
                musq = sp.tile([1, C], F32, tag="musq")
                nc.vector.tensor_tensor(musq[:], mu_sb.bitcast(F32)[:],
                                        mu_sb.bitcast(F32)[:], ALU.mult)
                nc.vector.tensor_tensor(var_sb[:], m2_ps[:], musq[:],
                                        ALU.subtract)
                sd_sb = sp.tile([1, C], F32, tag="sdsb")
                nc.scalar.activation(sd_sb[:], var_sb[:], AF.Sqrt,
                                     bias=eps_sb[:], scale=1.0)
                rstd_f = sp.tile([1, C], F32, tag="rstdf")
                nc.vector.reciprocal_approx_fast(rstd_f[:], sd_sb[:])
                rstd_sb = sp.tile([1, C], F32R, tag="rstdsb")
                nc.scalar.copy(rstd_sb[:], rstd_f[:])
                muex_ps = psT.tile([128, C], F32, tag="st", name=f"muex{ch}")
                nc.tensor.matmul(muex_ps[:], ok1_sb[:], mu_sb[:],
                                 start=True, stop=True)
                rsex_ps = psT.tile([128, C], F32, tag="st", name=f"rsex{ch}")
                nc.tensor.matmul(rsex_ps[:], ok1_sb[:], rstd_sb[:],
                                 start=True, stop=True)
                fused = ap.tile([128, 4, C], F32R, tag="fused")
                for p in range(4):
                    lnp = sp2.tile([128, C], F32, tag="lnp",
                                   name=f"lnp{ch}_{p}")
                    nc.vector.tensor_tensor(lnp[:], y_sb.bitcast(F32)[:, p, :],
                                            muex_ps[:], ALU.subtract)
                    nc.vector.tensor_tensor(lnp[:], lnp[:], rsex_ps[:],
                                            ALU.mult)
                    nc.scalar.activation(fused[:, p, :], lnp[:],
                                         AF.Identity,
                                         bias=lnb_sb[:, p:p + 1],
                                         scale=lng_sb[:, p:p + 1])

                # ---------------- routed expert matmul ----------------
                for m4 in range(4):
                    op = psH.tile([128, C], F32, tag="tail", name=f"op{ch}_{m4}")
                    for ks in range(4):
                        nc.tensor.matmul(
                            op[:],
                            waff_sb[:, ks, m4 * 128:(m4 + 1) * 128],
                            fused[:, ks, :], start=(ks == 0), stop=(ks == 3))
                    ot = sp2.tile([128, C], F32, tag="ot", name=f"ot{ch}_{m4}")
                    nc.scalar.activation(
                        ot[:], op[:], AF.Identity,
                        bias=baff_sb[:, exp_idx * 4 + m4:exp_idx * 4 + m4 + 1],
                        scale=1.0)
                    nc.sync.dma_start(outT[m4, :, col:col + C], ot[:])

    nc.finalize()
    return nc


def _prep_weights(inputs):
    in_proj_w = np.asarray(inputs["in_proj_w"], np.float32)
    in_proj_b = np.asarray(inputs["in_proj_b"], np.float32)
    out_proj_w = np.asarray(inputs["out_proj_w"], np.float32)
    out_proj_b = np.asarray(inputs["out_proj_b"], np.float32)
    fus_w1 = np.asarray(inputs["fus_w1"], np.float32)
    fus_b1 = np.asarray(inputs["fus_b1"], np.float32)
    fus_w2 = np.asarray(inputs["fus_w2"], np.float32)
    fus_b2 = np.asarray(inputs["fus_b2"], np.float32)
    ln_g = np.asarray(inputs["ln_g"], np.float32)
    ln_b = np.asarray(inputs["ln_b"], np.float32)
    aff_w = np.asarray(inputs["aff_w"], np.float32)
    aff_b = np.asarray(inputs["aff_b"], np.float32)

    scale = 1.0 / np.sqrt(np.float32(HD))
    W = in_proj_w.copy()
    W[:E] *= scale
    bq = in_proj_b.copy()
    bq[:E] *= scale
    # W.T is [512(k), 1536(m)]; sbuf wants [128, 4(ksub), 1536]
    wqkv_h = np.ascontiguousarray(
        W.T.reshape(4, 128, 1536).transpose(1, 0, 2))
    bqkv_h = np.ascontiguousarray(bq.reshape(12, 128).T)

    # fold out_proj into fus_w1; permute (h,d) -> (p, hl, d) to match v layout
    perm = np.empty(E, np.int64)
    for h in range(NH):
        for d in range(HD):
            perm[(h // 2) * 128 + (h % 2) * 64 + d] = h * HD + d
    blocks = []
    for i in range(3):
        blk = fus_w1[:, i * E:(i + 1) * E] @ out_proj_w  # [256, 512]
        blocks.append(blk[:, perm])
    W1o = np.concatenate(blocks, axis=1)  # [256, 1536] cols = (i, p, hl, d)
    w1o_h = np.ascontiguousarray(W1o.T.reshape(12, 128, 256).transpose(1, 0, 2))
    beff = fus_b1 + fus_w1 @ np.tile(out_proj_b, 3)
    beff_h = np.ascontiguousarray(beff.reshape(2, 128).T)

    w2_h = np.ascontiguousarray(fus_w2.T.reshape(2, 128, 512).transpose(1, 0, 2))
    b2_h = np.ascontiguousarray(fus_b2.reshape(4, 128).T)
    lng_h = np.ascontiguousarray(ln_g.reshape(4, 128).T)
    lnb_h = np.ascontiguousarray(ln_b.reshape(4, 128).T)

    A = np.concatenate([aff_w[e].T for e in range(NE)], axis=1)  # [512, 2048]
    waff_h = np.ascontiguousarray(A.reshape(4, 128, 2048).transpose(1, 0, 2))
    baff_h = np.ascontiguousarray(aff_b.reshape(NE * 4, 128).T)

    sel_h = np.zeros((128, 4, 8), np.float32)
    for r in range(128):
        for p in range(4):
            sel_h[r, p, 2 * p + r // 64] = 1.0
    exps_h = np.zeros((8, 4, 128), np.float32)
    for p in range(4):
        for c in range(128):
            exps_h[2 * p + c // 64, p, c] = 1.0

    bf = mybir.dt.np(BF16)
    return {
        "wqkv": wqkv_h, "bqkv": bqkv_h, "w1o": w1o_h, "beff": beff_h,
        "w2": w2_h, "b2": b2_h, "lng": lng_h, "lnb": lnb_h,
        "waff": waff_h, "baff": baff_h,
        "sel": sel_h.astype(bf), "exps": exps_h,
        "ones512": np.full((128, 1), 1.0 / E, np.float32),
        "onesk1": np.ones((1, 128), np.float32),
    }


def kernel(**inputs):
    img = np.asarray(inputs["image_embeddings"], np.float32)
    txt = np.asarray(inputs["text_embeddings"], np.float32)
    kno = np.asarray(inputs["knowledge_embeddings"], np.float32)
    labels = np.asarray(inputs["affective_labels"]).astype(np.int64).ravel()
    assert img.shape == (B, E)

    # ---- host-side expert routing (per-core fixed capacities) ----
    core_idx = np.zeros((NCORES, R), np.int64)
    core_val = np.zeros((NCORES, R), bool)
    for e in range(NE):
        ids = np.nonzero(labels == e)[0]
        assert len(ids) <= NCORES * CAP, f"expert {e} overflow: {len(ids)}"
        parts = np.array_split(ids, NCORES)
        for c in range(NCORES):
            seg = parts[c]
            core_idx[c, e * CAP: e * CAP + len(seg)] = seg
            core_val[c, e * CAP: e * CAP + len(seg)] = True

    wmap = _prep_weights(inputs)

    in_maps = []
    for c in range(NCORES):
        gi = core_idx[c]
        xg = np.stack([img[gi], txt[gi], kno[gi]])        # [3, R, 512]
        xg = xg.transpose(0, 2, 1)                        # [3, 512, R]
        xt_h = np.ascontiguousarray(xg.reshape(3, 4, 128, R))
        m = dict(wmap)
        m["xt"] = xt_h
        in_maps.append(m)

    nc = _build_program()
    res = run_bass_kernel_spmd(nc, in_maps, core_ids=list(range(NCORES)))
    global LAST_RESULTS, LAST_NC
    LAST_RESULTS = res
    LAST_NC = nc

    out_full = np.zeros((B, E), np.float32)
    for c in range(NCORES):
        oT = res.results[c]["outT"].reshape(E, R).T       # [R, 512]
        v = core_val[c]
        out_full[core_idx[c][v]] = oT[v]
    return out_full


if __name__ == "__main__":
    rng = np.random.default_rng(0)
    fake = {
        "image_embeddings": rng.standard_normal((B, E)).astype(np.float32),
        "text_embeddings": rng.standard_normal((B, E)).astype(np.float32),
        "knowledge_embeddings": rng.standard_normal((B, E)).astype(np.float32),
        "affective_labels": rng.integers(0, NE, B),
        "in_proj_w": (rng.standard_normal((3 * E, E)) * 0.02).astype(np.float32),
        "in_proj_b": np.zeros(3 * E, np.float32),
        "out_proj_w": (rng.standard_normal((E, E)) * 0.02).astype(np.float32),
        "out_proj_b": np.zeros(E, np.float32),
        "fus_w1": (rng.standard_normal((H, 3 * E)) * 0.02).astype(np.float32),
        "fus_b1": np.zeros(H, np.float32),
        "fus_w2": (rng.standard_normal((E, H)) * 0.02).astype(np.float32),
        "fus_b2": np.zeros(E, np.float32),
        "ln_g": np.ones(E, np.float32),
        "ln_b": np.zeros(E, np.float32),
        "aff_w": (rng.standard_normal((NE, E, E)) * 0.02).astype(np.float32),
        "aff_b": np.zeros((NE, E), np.float32),
    }
    out = kernel(**fake)
    print("kernel ran, out:", out.shape, out.dtype, np.abs(out).max())
